# revision 28
# baseline (speedup 1.0000x reference)
"""Bass/Trainium2 kernel for nn_MultiHeadAttentionBlock_23502061043960.

Reference math (note: the module multiplies RAW scores with value — no
softmax in the output path — so the whole block is linear):

    out = (concat_h Q_h (K_h^T V_h) / 8) @ w_o.T + b_o
        where Q = q w_q^T, K = k w_k^T, V = v w_v^T   (biases are zero)

Linearity lets us contract the sequence dim first and never materialize
the [B,H,S,S] score tensor:

    A_b    = k_b^T v_b                     [512, 512]   (per batch)
    M_h    = w_k[h] A_b w_v[h]^T / 8       [64, 64]     (per head)
    W2^T   = blockdiag(M_h) w_o^T          [512, 512]
    Wfold  = w_q^T W2^T                    [512, 512]
    out_b  = q_b Wfold + b_o               (one dense matmul per row)

Sharding over 8 cores (2 batches x 2 A-halves x 2 q-halves): A and the
whole fold chain are LINEAR in A, so a core can contract only HALF of
the sequence rows of k/v into a partial A, fold that into a partial
Wfold, and apply it to half of the q rows; the host then SUMS the two
partial outputs per q-row block (a plain sum-unshard, the same reduce a
row-sharded w_o would need). Versus the previous all-cores-redundant-A
version this halves per-core A matmul work (27->14 us) and k/v DMA
(8->4 MB) at the cost of doubling the apply (7->14 us), a large net win
because A's DMA and PE cost dominated. No collectives: on this stack
any collective drags in an all-core start barrier absorbing tens of us
of inter-core launch skew.

Layout/staging choices (all host-side, free at HW time):
 - q is staged as q^T and the output returned as out^T, because the PE
   array contracts over the partition dim;
 - weights are staged transposed so they can be the stationary operand;
 - k/v are staged "pair-packed" ([128, 1024] tiles: two row-chunks
   side by side) so each DMA partition row is a 2 KiB contiguous run;
   weights are quad-packed the same way;
 - the 1/sqrt(dk) = 1/8 score scale is folded into the staged w_k;
 - b_o is added on the host during the partial-output sum (free).

W2^T is built with two concurrent 64-row/64-col quadrant matmuls per
128-row chunk (tile_position row/col groups), skipping the previous
memset + SBUF->SBUF-DMA blockdiag construction (~3 us of latency).

dtype: all matmul inputs fp16 (host-staged; same 2-byte DMA cost and
full 1-cycle/row PE rate as bf16 but 10 mantissa bits), fp32 PSUM
accumulation throughout, fp16 output upcast + summed on host. All
intermediates are bounded well under fp16 range.
"""

import ml_dtypes
import numpy as np

import concourse.mybir as mybir
import concourse.tile as tile
from concourse import bacc
from concourse.bass_utils import run_bass_kernel_spmd

B = 2
S = 4096
D = 512
H = 8
DK = 64
N_CORES = 8
SH = S // 2  # 2048 k/v rows per core (A contraction half)
SQ = S // 2  # 2048 q/out rows per core
P = 128
F32 = mybir.dt.float32

USE_BF16 = True

_compiled = {}

LAST_RESULTS = None  # test harness reads exec_time_ns / trace from here
RUN_KW = {}  # test harness can inject trace kwargs


def _build():
    nc = bacc.Bacc()

    DT = mybir.dt.float16 if USE_BF16 else mybir.dt.float32r

    # k/v pair-packed: 2 row-chunks of [128, 512] side by side per
    # [128, 1024] tile -> every DMA partition row is a 2 KiB run.
    kb = nc.declare_dram_parameter("kb", [SH // 2, 2 * D], DT, isOutput=False)
    vb = nc.declare_dram_parameter("vb", [SH // 2, 2 * D], DT, isOutput=False)
    qT = nc.declare_dram_parameter("qT", [D, SQ], DT, isOutput=False)
    wkT = nc.declare_dram_parameter("wkT", [P, 4 * D], DT, isOutput=False)
    wvT = nc.declare_dram_parameter("wvT", [P, 4 * D], DT, isOutput=False)
    wq = nc.declare_dram_parameter("wq", [P, 4 * D], DT, isOutput=False)
    woT = nc.declare_dram_parameter("woT", [P, 4 * D], DT, isOutput=False)
    outT = nc.declare_dram_parameter("outT", [D, SQ], DT, isOutput=True)

    kb_v = kb.rearrange("(n p) d -> n p d", p=P)  # 8 x [128, 1024]
    vb_v = vb.rearrange("(n p) d -> n p d", p=P)
    qT_v = qT.rearrange("(n p) d -> n p d", p=P)  # 4 x [128, 2048]
    outT_v = outT.rearrange("(n p) d -> n p d", p=P)  # 4 x [128, 2048]

    NKC = SH // P  # 16 contraction chunks for A
    NDC = D // P  # 4 chunks of the model dim
    NG = NKC // 2  # 8 pair-packed k/v tiles
    NN = SQ // D  # 4 output column groups per model-dim chunk

    with tile.TileContext(nc) as tc:
        with (
            tc.tile_pool(name="w", bufs=1) as wp,
            tc.tile_pool(name="kv", bufs=1) as kvp,
            tc.tile_pool(name="qt", bufs=1) as qtp,
            tc.tile_pool(name="work", bufs=NDC) as wkpool,
            tc.tile_pool(name="small", bufs=1) as smallp,
            tc.tile_pool(name="ot", bufs=4) as otp,
            tc.tile_pool(name="psB", bufs=3, space="PSUM") as psb,
            tc.tile_pool(name="psD", bufs=1, space="PSUM") as psd,
        ):
            # Scratch tiles for PE warm-up / warm-keeper matmuls: the HAM
            # clock gate starts the PE at 1.2 GHz and re-throttles after
            # idle windows; junk matmuls during the startup barrier and the
            # cast-bound fold gaps keep the array at 2.4 GHz. Contents are
            # garbage; the results are never read.
            # GpSimd finishes its framework preamble first (~6 us), so it
            # initializes the warm tile; small N=128 warm matmuls then run
            # from ~6.3 us so HAM un-throttles before the first k/v chunk
            # lands and phase 1 never runs at 1.2 GHz.
            warm_sb = wp.tile([P, P], DT, name="warm", tag="warm")
            warm_ps = psd.tile([P, D], F32, name="warmps", tag="warmps")
            nc.gpsimd.memset(warm_sb[:].bitcast(mybir.dt.uint32), 0)

            def warm(n, nfree=P):
                for _ in range(n):
                    nc.tensor.matmul(warm_ps[:, 0:nfree], warm_sb[:], warm_sb[:, 0:nfree], start=True, stop=True)

            warm(24)  # runs during the entry barrier + DMA ramp

            # psA lives only for phase 1; closing it lets psW reuse its
            # banks (PSUM is 8 banks total: 4 psA / 3 psB / 4 psW / 1 warm).
            a_sb = []
            with tc.tile_pool(name="psA", bufs=NDC, space="PSUM") as psa:
                # ---- phase 1: A = k^T v, streaming k/v chunk pairs -------
                # loads and matmuls interleaved: the PE chases the DMA stream
                a_ps = [psa.tile([P, D], F32, name=f"aps{m}", tag="aps") for m in range(NDC)]
                # first two pairs as standalone chunk tiles so the early
                # matmuls chase 0.125 MiB arrivals through the DMA ramp-up.
                # k streams on the Sync HWDGE ring, v on the Scalar one:
                # two descriptor-generation rings halve the ramp-up and the
                # per-pair arrival latency (HBM bandwidth is shared anyway)
                NSPLIT = 2
                k0 = [kvp.tile([P, D], DT, name=f"k0{j}", tag=f"k0{j}") for j in range(2 * NSPLIT)]
                v0 = [kvp.tile([P, D], DT, name=f"v0{j}", tag=f"v0{j}") for j in range(2 * NSPLIT)]
                k_t = [kvp.tile([P, 2 * D], DT, name=f"k{i}", tag=f"k{i}") for i in range(NSPLIT, NG)]
                v_t = [kvp.tile([P, 2 * D], DT, name=f"v{i}", tag=f"v{i}") for i in range(NSPLIT, NG)]
                for j in range(2 * NSPLIT):
                    g, jj = divmod(j, 2)
                    js = slice(jj * D, (jj + 1) * D)
                    nc.sync.dma_start(out=k0[j][:], in_=kb_v[g][:, js])
                    nc.scalar.dma_start(out=v0[j][:], in_=vb_v[g][:, js])
                    for m in range(NDC):
                        nc.tensor.matmul(
                            a_ps[m][:],
                            k0[j][:, m * P : (m + 1) * P],
                            v0[j][:],
                            start=(j == 0),
                            stop=False,
                        )
                for g in range(NSPLIT, NG):
                    nc.sync.dma_start(out=k_t[g - NSPLIT][:], in_=kb_v[g])
                    nc.scalar.dma_start(out=v_t[g - NSPLIT][:], in_=vb_v[g])
                    last = g == NG - 1
                    # last pair runs m-outer so a_ps[0] stops ~1.3 us before
                    # the A tail ends and its cast + F1 start that much sooner
                    order = (
                        [(j, m) for m in range(NDC) for j in range(2)]
                        if last
                        else [(j, m) for j in range(2) for m in range(NDC)]
                    )
                    for j, m in order:
                        nc.tensor.matmul(
                            a_ps[m][:],
                            k_t[g - NSPLIT][:, j * D + m * P : j * D + (m + 1) * P],
                            v_t[g - NSPLIT][:, j * D : (j + 1) * D],
                            start=False,
                            stop=(last and j == 1),
                        )

                # ---- remaining loads queue strictly behind the k/v stream
                # (anything ahead of the last k/v pair delays the A tail);
                # fold weights first (needed ~5 us before q), split across
                # both rings ---------------------------------------------
                wk_t = wp.tile([P, 4 * D], DT, name="wkt", tag="wkt")
                wv_t = wp.tile([P, 4 * D], DT, name="wvt", tag="wvt")
                wq_t = wp.tile([P, 4 * D], DT, name="wqt", tag="wqt")
                wo_t = wp.tile([P, 4 * D], DT, name="wot", tag="wot")
                # tail loads balanced across both rings (4 MB each side);
                # a lone ring hauling all 4 MB would gate the apply on qT
                nc.sync.dma_start(out=wk_t[:], in_=wkT[:])
                nc.scalar.dma_start(out=wv_t[:], in_=wvT[:])
                nc.sync.dma_start(out=wq_t[:], in_=wq[:])
                nc.scalar.dma_start(out=wo_t[:], in_=woT[:])
                qt_t = [qtp.tile([P, SQ], DT, name=f"q{i}", tag=f"q{i}") for i in range(NDC)]
                for i in range(NDC):
                    eng = nc.sync if i % 2 == 0 else nc.scalar
                    eng.dma_start(out=qt_t[i][:], in_=qT_v[i])

                # PSUM->SBUF casts split across Vector/Scalar/GpSimd and
                # into half-tiles, low halves first: F1's kc=0/1 column
                # blocks are ready after the first four quarter-size casts,
                # so F1 starts ~0.4 us after A's accumulation stops instead
                # of waiting ~2.8 us for four full-tile casts.
                # (GpSimd/Pool cannot read PSUM — neuronx-cc rejects it —
                # so only Vector and Scalar serve as cast lanes)
                cast_lanes = [nc.vector.tensor_copy, nc.scalar.copy]
                a_sb = [wkpool.tile([P, D], DT, name="a", tag="a") for _ in range(NDC)]
                for h in range(2):
                    hs = slice(h * (D // 2), (h + 1) * (D // 2))
                    for m in range(NDC):
                        cast_lanes[m % 2](a_sb[m][:, hs], a_ps[m][:, hs])

            with tc.tile_pool(name="psW", bufs=4, space="PSUM") as psw:
                # ---- fold F1+F2, chunk-pipelined: F2 (band of G = w_v Y^T,
                # whose diag blocks are M_h^T) accumulates over kc chunks, so
                # each F1 output chunk feeds F2 as soon as it is copied.
                mT = smallp.tile([P, D], DT, name="mT", tag="mT")
                g_ps = [psw.tile([P, P], F32, name=f"gps{m}", tag="pw") for m in range(NDC)]

                def f2_group(kc, yT):
                    for mp in range(NDC):
                        nc.tensor.matmul(
                            g_ps[mp][:],
                            wv_t[:, kc * D + mp * P : kc * D + (mp + 1) * P],
                            yT[:, mp * P : (mp + 1) * P],
                            start=(kc == 0),
                            stop=(kc == NDC - 1),
                        )

                # F2 groups are emitted one kc behind F1, so each F2 group's
                # yT cast completes while the PE runs the next F1 group
                yts = []
                for kc in range(NDC):
                    y_ps = psb.tile([P, D], F32, name="yps", tag="ps")
                    for kd in range(NDC):
                        nc.tensor.matmul(
                            y_ps[:],
                            a_sb[kd][:, kc * P : (kc + 1) * P],
                            wk_t[:, kd * D : (kd + 1) * D],
                            start=(kd == 0),
                            stop=(kd == NDC - 1),
                        )
                    yT = wkpool.tile([P, D], DT, name="yT", tag="yT")
                    cast_lanes[kc % 2](yT[:], y_ps[:])
                    yts.append(yT)
                    if kc >= 1:
                        f2_group(kc - 1, yts[kc - 1])
                f2_group(NDC - 1, yts[NDC - 1])
                # mT[64*(h%2):, h*64:] = M_h^T (the diag 64-blocks of G);
                # partition-aligned copies, alternating Scalar/Vector (the
                # 1/8 scale is already folded into the staged wkT)
                for m in range(NDC):
                    for hh in range(2):  # heads 2m, 2m+1
                        h = 2 * m + hh
                        cast_lanes[(m + 1) % 2](
                            mT[hh * DK : (hh + 1) * DK, h * DK : (h + 1) * DK],
                            g_ps[m][hh * DK : (hh + 1) * DK, hh * DK : (hh + 1) * DK],
                        )

                # ---- phase 2b: W2^T = BD(M) w_o^T, two concurrent 64-row
                # quadrant matmuls per 128-row chunk (heads 2p, 2p+1) ------
                w2_sb = []
                for p in range(NDC):
                    w2_ps = psb.tile([P, D], F32, name="w2ps", tag="ps")
                    for hh in range(2):
                        h = 2 * p + hh
                        hs = slice(hh * DK, (hh + 1) * DK)
                        nc.tensor.matmul(
                            w2_ps[hs, :],
                            mT[hs, h * DK : (h + 1) * DK],
                            wo_t[hs, p * D : (p + 1) * D],
                            start=True,
                            stop=True,
                        )
                    t = wkpool.tile([P, D], DT, name="w2", tag="w2")
                    eng = nc.vector.tensor_copy if p % 2 == 0 else nc.scalar.copy
                    eng(t[:], w2_ps[:])
                    w2_sb.append(t)

                # ---- fold Wfold = w_q^T W2^T  (out = q Wfold) ------------
                wf_sb = []
                for m in range(NDC):
                    wf_ps = psb.tile([P, D], F32, name="wfps", tag="ps")
                    for kc in range(NDC):
                        nc.tensor.matmul(
                            wf_ps[:],
                            wq_t[:, kc * D + m * P : kc * D + (m + 1) * P],
                            w2_sb[kc][:],
                            start=(kc == 0),
                            stop=(kc == NDC - 1),
                        )
                    t = wkpool.tile([P, D], DT, name="wf", tag="wf")
                    eng = nc.vector.tensor_copy if m % 2 == 0 else nc.scalar.copy
                    eng(t[:], wf_ps[:])
                    wf_sb.append(t)

                # ---- phase 2c: out^T = Wfold^T q^T (bias added on host) --
                for m in range(NDC):
                    o_sb = otp.tile([P, SQ], DT, name="osb", tag="osb")
                    for nn in range(NN):
                        ns = slice(nn * D, (nn + 1) * D)
                        o_ps = psw.tile([P, D], F32, name="ops", tag="pw")
                        for kc in range(NDC):
                            nc.tensor.matmul(
                                o_ps[:],
                                wf_sb[kc][:, m * P : (m + 1) * P],
                                qt_t[kc][:, ns],
                                start=(kc == 0),
                                stop=(kc == NDC - 1),
                            )
                        # odd nn on vector so the last cast before each
                        # store is on the faster engine
                        eng = nc.vector.tensor_copy if nn % 2 == 1 else nc.scalar.copy
                        eng(o_sb[:, ns], o_ps[:])
                        if nn % 2 == 1:  # store per nn-pair: shorter tail
                            hs = slice((nn - 1) * D, (nn + 1) * D)
                            nc.sync.dma_start(out=outT_v[m][:, hs], in_=o_sb[:, hs])

    nc.compile()
    return nc


def kernel(q, k, v, w_q, b_q, w_k, b_k, w_v, b_v, w_o, b_o):
    global LAST_RESULTS
    key = ("nc", USE_BF16)
    if key not in _compiled:
        _compiled[key] = _build()
    nc = _compiled[key]

    np_dt = np.float16 if USE_BF16 else np.float32

    def packn(x, w):  # [N, 512] -> [N//w, w*512]: w row-chunks side by side
        n = x.shape[0] // (w * P)
        return np.ascontiguousarray(
            x.reshape(n, w, P, D).transpose(0, 2, 1, 3).reshape(n * P, w * D)
        )

    def pack4(x):
        return packn(x, 4)

    q = np.asarray(q, dtype=np.float32)
    k = np.asarray(k, dtype=np.float32)
    v = np.asarray(v, dtype=np.float32)
    # per (batch, half): pair-packed k/v halves
    kc_ = [
        [packn(k[b, i * SH : (i + 1) * SH].astype(np_dt), 2) for i in range(2)]
        for b in range(B)
    ]
    vc_ = [
        [packn(v[b, i * SH : (i + 1) * SH].astype(np_dt), 2) for i in range(2)]
        for b in range(B)
    ]
    qt_ = [
        [
            np.ascontiguousarray(q[b, j * SQ : (j + 1) * SQ, :].T).astype(np_dt)
            for j in range(2)
        ]
        for b in range(B)
    ]
    wkT = pack4((np.asarray(w_k, np.float32).T * 0.125).astype(np_dt))
    wvT = pack4(np.asarray(w_v, np.float32).T.astype(np_dt))
    wqn = pack4(np.asarray(w_q, np.float32).astype(np_dt))
    woT = pack4(np.asarray(w_o, np.float32).T.astype(np_dt))

    in_maps = []
    for c in range(N_CORES):
        b, ij = divmod(c, 4)
        i, j = divmod(ij, 2)
        in_maps.append(
            {
                "kb": kc_[b][i],
                "vb": vc_[b][i],
                "qT": qt_[b][j],
                "wkT": wkT,
                "wvT": wvT,
                "wq": wqn,
                "woT": woT,
            }
        )

    res = run_bass_kernel_spmd(nc, in_maps, list(range(N_CORES)), **RUN_KW)
    LAST_RESULTS = res

    bo = np.asarray(b_o, np.float32)
    out = np.empty((B, S, D), dtype=np.float32)
    for b in range(B):
        for j in range(2):
            c0 = b * 4 + 0 * 2 + j  # A-half 0
            c1 = b * 4 + 1 * 2 + j  # A-half 1
            part = res.results[c0]["outT"].astype(np.float32) + res.results[c1][
                "outT"
            ].astype(np.float32)
            out[b, j * SQ : (j + 1) * SQ, :] = part.T + bo
    return out
